# revision 7
# baseline (speedup 1.0000x reference)
"""Trainium2 Bass kernel for the counting-criterion loss.

Computes, for output/density_map of shape [32, 1, 512, 512] and bboxes [32, 3, 4]:
  dmap_loss  = sum((output - density_map)^2) / num_objects
  count_loss = mean_b((sum(output_b) - sum(density_map_b))^2)
  min_count  = sum_boxes(relu(1 - box_sum))   with box sums over [y1:y2, x1:x2)

Strategy: data-parallel over the batch — core i handles images [4i, 4i+4).
The stream is ordered so the post-stream tail is minimal: images 0-2 as
o/d half pairs (diff on DVE, square on ACT, box sums on PE), then image 3's
o quarters interleaved with its d pieces, the d pieces tapering down to
128-column chunks so the last-arrival diff+square chain on DVE is as short
as possible.  Final tiny reductions run on the host from each core's
[128, NCOLS] accumulator.
"""

import numpy as np
from contextlib import ExitStack

import concourse.bass as bass
import concourse.mybir as mybir
import concourse.tile as tile
from concourse import bacc
from concourse.bass_utils import run_bass_kernel_spmd

N_CORES = 8
B, H, W = 32, 512, 512
NIMG = B // N_CORES  # images per core
P = 128              # SBUF partitions
NCH = H // P         # row chunks per image (and col chunks: W//P)
NB = 3               # boxes per image
F32 = mybir.dt.float32
U8 = mybir.dt.uint8

# image-3 stream: ("o", row_chunk) quarters interleaved with d pieces
# ("d", row_chunk, x0, x1, square_engine), tapering to 128-col chunks so the
# final diff+square chain after the last arrival is short.
IMG3_STREAM = [
    ("o", 0),
    ("d", 0, 0, 512, "act"),
    ("o", 1),
    ("d", 1, 0, 512, "act"),
    ("o", 2),
    ("d", 2, 0, 256, "act"),
    ("o", 3),
    ("d", 2, 256, 384, "act"),
    ("d", 2, 384, 512, "act"),
    ("d", 3, 0, 128, "act"),
    ("d", 3, 128, 256, "dve"),
    ("d", 3, 256, 384, "dve"),
    ("d", 3, 384, 512, "dve"),
]
ND3 = sum(1 for e in IMG3_STREAM if e[0] == "d")  # 9

# accumulator columns: per-chunk sum(o-d) | per-chunk sum((o-d)^2) | box partials
NDIFF = 2 * (NIMG - 1) + ND3               # 6 + 9 = 15
NBOXCOL = NIMG * NCH * NB                  # 48
NCOLS = 2 * NDIFF + NBOXCOL                # 78
MW = 2 * NIMG * NCH * NB                   # 96 mask columns

_PROG = None


def _build_program():
    nc = bacc.Bacc(
        "TRN2",
        target_bir_lowering=False,
        debug=False,
        num_devices=N_CORES,
    )
    o_d = nc.dram_tensor("o", [NIMG, H, W], F32, kind="ExternalInput").ap()
    d_d = nc.dram_tensor("d", [NIMG, H, W], F32, kind="ExternalInput").ap()
    # packed uint8 masks: cols 0:48 row mask [(img, c, j)], 48:96 col mask
    msk_d = nc.dram_tensor("msk", [P, MW], U8, kind="ExternalInput").ap()
    acc_d = nc.dram_tensor("acc", [P, NCOLS], F32, kind="ExternalOutput").ap()

    # DRAM views: image rows split as y = c*128 + p  ->  [img, p, c, x]
    o_r = o_d.rearrange("n (c p) x -> n p c x", p=P)
    d_r = d_d.rearrange("n (c p) x -> n p c x", p=P)

    with tile.TileContext(nc) as tc, ExitStack() as ctx:
        io_pool = ctx.enter_context(tc.tile_pool(name="io", bufs=2))
        o3_pool = ctx.enter_context(tc.tile_pool(name="o3", bufs=1))
        d3_pool = ctx.enter_context(tc.tile_pool(name="d3", bufs=1))
        mask_pool = ctx.enter_context(tc.tile_pool(name="mask", bufs=1))
        work_pool = ctx.enter_context(tc.tile_pool(name="work", bufs=2))
        psum_pool = ctx.enter_context(tc.tile_pool(name="psum", bufs=2, space="PSUM"))
        acc_pool = ctx.enter_context(tc.tile_pool(name="acc", bufs=1))

        acc = acc_pool.tile([P, NCOLS], F32)
        nc.vector.memset(acc[:], 0.0)
        ones_t = acc_pool.tile([P, 1], F32)
        nc.vector.memset(ones_t[:], 1.0)
        msk_u8 = mask_pool.tile([P, MW], U8)
        msk_f = mask_pool.tile([P, MW], F32)

        def rm(img, cy):
            c0 = img * NCH * NB + cy * NB
            return msk_f[:, c0 : c0 + NB]

        def cm(img):
            c0 = NIMG * NCH * NB + img * NCH * NB
            return msk_f[:, c0 : c0 + NCH * NB]

        def box_work(img, chunk):
            """chunk(cy, xs) -> [128, |xs|] AP of o rows cy*128+p, cols xs."""
            ps = psum_pool.tile([P, NCH * NB], F32, tag="ps")
            for cx in range(NCH):
                for cy in range(NCH):
                    nc.tensor.matmul(
                        ps[:, cx * NB : (cx + 1) * NB],
                        lhsT=chunk(cy, slice(cx * P, (cx + 1) * P)),
                        rhs=rm(img, cy),
                        start=(cy == 0),
                        stop=(cy == NCH - 1),
                    )
            masked_t = work_pool.tile([P, NCH * NB], F32, tag="masked")
            nc.vector.tensor_tensor(
                out=masked_t[:], in0=ps[:], in1=cm(img), op=mybir.AluOpType.mult
            )
            ps2 = psum_pool.tile([1, NCH * NB], F32, tag="ps2")
            nc.tensor.matmul(
                ps2[:], lhsT=ones_t[:], rhs=masked_t[:], start=True, stop=True
            )
            col0 = 2 * NDIFF + img * NCH * NB
            nc.vector.tensor_copy(acc[0:1, col0 : col0 + NCH * NB], ps2[:])

        def diff_work(o_ap, d_ap, col, sq_eng, tag, bufs=None):
            """stt diff (DVE) + square, each accumulating into acc columns."""
            shp = list(o_ap.shape)
            diff_t = work_pool.tile(shp, F32, tag="diff" + tag, bufs=bufs)
            nc.vector.scalar_tensor_tensor(
                out=diff_t[:],
                in0=o_ap,
                scalar=0.0,
                in1=d_ap,
                op0=mybir.AluOpType.bypass,
                op1=mybir.AluOpType.subtract,
                accum_out=acc[:, col : col + 1],
            )
            sq_t = work_pool.tile(shp, F32, tag="sq" + tag, bufs=bufs)
            scol = NDIFF + col
            eng = {"dve": nc.vector, "pool": nc.gpsimd}.get(sq_eng)
            if eng is not None:
                eng.scalar_tensor_tensor(
                    out=sq_t[:],
                    in0=diff_t[:],
                    scalar=0.0,
                    in1=diff_t[:],
                    op0=mybir.AluOpType.bypass,
                    op1=mybir.AluOpType.mult,
                    accum_out=acc[:, scol : scol + 1],
                )
            else:
                nc.scalar.activation(
                    sq_t[:],
                    diff_t[:],
                    mybir.ActivationFunctionType.Square,
                    accum_out=acc[:, scol : scol + 1],
                )

        # ---- images 0..2: half-image o/d pairs ----
        HC = NCH // 2
        for img in range(NIMG - 1):
            halves = []
            for h in range(2):
                o_t = io_pool.tile([P, HC, W], F32, tag=f"o{h}")
                nc.sync.dma_start(o_t[:], o_r[img, :, h * HC : (h + 1) * HC])
                if img == 0 and h == 0:
                    # small mask DMA tucked behind the first transfer
                    nc.sync.dma_start(msk_u8[:], msk_d)
                d_t = io_pool.tile([P, HC, W], F32, tag=f"d{h}")
                nc.sync.dma_start(d_t[:], d_r[img, :, h * HC : (h + 1) * HC])
                if img == 0 and h == 0:
                    nc.vector.tensor_copy(msk_f[:], msk_u8[:])
                diff_work(o_t[:], d_t[:], 2 * img + h, "act", tag="h")
                halves.append(o_t)
            box_work(img, lambda cy, xs: halves[cy // HC][:, cy % HC, xs])

        # ---- image 3: o quarters interleaved with tapered d pieces ----
        img = NIMG - 1
        o3 = {}
        d3 = {
            c: d3_pool.tile([P, 1, W], F32, tag=f"d3c{c}", name=f"d3c{c}")
            for c in range(NCH)
        }
        di = 0
        for entry in IMG3_STREAM:
            if entry[0] == "o":
                c = entry[1]
                o3[c] = o3_pool.tile([P, 1, W], F32, tag=f"o3c{c}", name=f"o3c{c}")
                nc.sync.dma_start(o3[c][:], o_r[img, :, c : c + 1])
                if c == NCH - 1:
                    # all of o3 resident: box sums for image 3 run here,
                    # well before the final d pieces arrive
                    box_work(img, lambda cy, xs: o3[cy][:, 0, xs])
            else:
                _, c, x0, x1, sq_eng = entry
                nc.sync.dma_start(d3[c][:, 0, x0:x1], d_r[img, :, c, x0:x1])
                diff_work(
                    o3[c][:, 0, x0:x1],
                    d3[c][:, 0, x0:x1],
                    2 * (NIMG - 1) + di,
                    sq_eng,
                    tag=f"q{x1 - x0}",
                    bufs=4,
                )
                di += 1

        nc.sync.dma_start(acc_d, acc[:])

    nc.compile()
    return nc


def _get_program():
    global _PROG
    if _PROG is None:
        _PROG = _build_program()
    return _PROG


def _prep_inputs(output, density_map, bboxes):
    o = np.ascontiguousarray(np.asarray(output, dtype=np.float32).reshape(B, H, W))
    dm = np.ascontiguousarray(
        np.asarray(density_map, dtype=np.float32).reshape(B, H, W)
    )
    bb = np.clip(np.asarray(bboxes).astype(np.int64), 0, W).astype(np.int32)
    x1, y1, x2, y2 = bb[..., 0], bb[..., 1], bb[..., 2], bb[..., 3]
    x2 = np.maximum(x2, x1)
    y2 = np.maximum(y2, y1)

    ar = np.arange(H, dtype=np.int32)
    # rm[b, y, j] = 1 if y1 <= y < y2, packed as [b, y%128, (y//128, j)]
    rmk = (
        (ar[None, :, None] >= y1[:, None, :]) & (ar[None, :, None] < y2[:, None, :])
    ).astype(np.uint8)
    rmk = rmk.reshape(B, NCH, P, NB).transpose(0, 2, 1, 3)  # [B, P, NCH, NB]
    # cm[b, j, x] = 1 if x1 <= x < x2, packed as [b, x%128, (x//128, j)]
    cmk = (
        (ar[None, None, :] >= x1[:, :, None]) & (ar[None, None, :] < x2[:, :, None])
    ).astype(np.uint8)
    cmk = cmk.reshape(B, NB, NCH, P).transpose(0, 3, 2, 1)  # [B, P, NCH, NB]
    # per-core mask tensor [P, 96]: [(img, c, j)] row-mask | col-mask blocks
    rmk = rmk.reshape(N_CORES, NIMG, P, NCH * NB).transpose(0, 2, 1, 3)
    cmk = cmk.reshape(N_CORES, NIMG, P, NCH * NB).transpose(0, 2, 1, 3)
    msk = np.ascontiguousarray(
        np.concatenate(
            [rmk.reshape(N_CORES, P, -1), cmk.reshape(N_CORES, P, -1)], axis=2
        )
    )  # [N_CORES, P, 96] uint8
    return o, dm, msk


def kernel(output, density_map, bboxes, num_objects):
    o, dm, msk = _prep_inputs(output, density_map, bboxes)

    nc = _get_program()
    in_maps = [
        {
            "o": o[i * NIMG : (i + 1) * NIMG],
            "d": dm[i * NIMG : (i + 1) * NIMG],
            "msk": msk[i],
        }
        for i in range(N_CORES)
    ]
    res = run_bass_kernel_spmd(nc, in_maps, core_ids=list(range(N_CORES)))

    per_img_d = []
    sq_total = 0.0
    box_sums = []
    for r in res.results:
        a = r["acc"]
        dcols = a[:, :NDIFF].sum(axis=0, dtype=np.float64)          # [15]
        per_img = [dcols[2 * i] + dcols[2 * i + 1] for i in range(NIMG - 1)]
        per_img.append(dcols[2 * (NIMG - 1) :].sum())
        per_img_d.extend(per_img)
        sq_total += a[:, NDIFF : 2 * NDIFF].sum(dtype=np.float64)
        box_sums.append(
            a[0, 2 * NDIFF :]
            .reshape(NIMG, NCH, NB)
            .sum(axis=1, dtype=np.float64)
            .reshape(-1)
        )
    per_img_d = np.array(per_img_d)
    box_sums = np.concatenate(box_sums)

    dmap_loss = sq_total / float(num_objects)
    count_loss = float(np.mean(per_img_d**2))
    min_count = float(np.maximum(0.0, 1.0 - box_sums).sum())
    return np.array([dmap_loss, count_loss, min_count], dtype=np.float32)


# revision 8
# speedup vs baseline: 1.0295x; 1.0295x over previous
"""Trainium2 Bass kernel for the counting-criterion loss.

Computes, for output/density_map of shape [32, 1, 512, 512] and bboxes [32, 3, 4]:
  dmap_loss  = sum((output - density_map)^2) / num_objects
  count_loss = mean_b((sum(output_b) - sum(density_map_b))^2)
  min_count  = sum_boxes(relu(1 - box_sum))   with box sums over [y1:y2, x1:x2)

Strategy: data-parallel over the batch — core i handles images [4i, 4i+4).
Stream order minimizes the post-stream tail: images 0-2 as o/d half pairs
(diff on DVE, square on ACT, box sums on PE), then image 3's o quarters (a
low-work zone that lets both vector engines drain), then image 3's d in
pieces tapering to 256 columns; the last two squares run on DVE so ACT's
expensive accumulator-read ops stay off the critical chain.  The [128, 78]
accumulator is written back by a SWDGE kv_writeback descriptor prepared
early and fired by trigger_dma at the end — the post-compute cost is just
the trigger + transfer + semaphore, not a full HWDGE DMA setup.
Final tiny reductions run on the host.
"""

import numpy as np
from contextlib import ExitStack

import concourse.bass as bass
import concourse.mybir as mybir
import concourse.tile as tile
from concourse import bacc
from concourse.bass_utils import run_bass_kernel_spmd

N_CORES = 8
B, H, W = 32, 512, 512
NIMG = B // N_CORES  # images per core
P = 128              # SBUF partitions
NCH = H // P         # row chunks per image (and col chunks: W//P)
NB = 3               # boxes per image
F32 = mybir.dt.float32
U8 = mybir.dt.uint8

# image-3 d pieces: (row_chunk, x0, x1, square_engine)
D3_PIECES = [
    (0, 0, 512, "act"),
    (1, 0, 512, "act"),
    (2, 0, 512, "act"),
    (3, 0, 256, "dve"),
    (3, 256, 512, "dve"),
]

# accumulator columns: per-chunk sum(o-d) | per-chunk sum((o-d)^2) | box partials
NDIFF = 2 * (NIMG - 1) + len(D3_PIECES)    # 6 + 5 = 11
NBOXCOL = NIMG * NCH * NB                  # 48
NCOLS = 2 * NDIFF + NBOXCOL                # 70
MW = 2 * NIMG * NCH * NB                   # 96 mask columns

_PROG = None


def _emit_wb_prep(nc, out_ap, in_ap, ctx_idxs_ap, queue_num=0):
    """bass.BassGpSimd.kv_writeback(prepare_only=True) without a user DMA
    sem, so Tile's DMASW bookkeeping owns completion (passing a user sem
    leaves Tile's DMASW lane semaphore un-incremented -> deadlock)."""
    g = nc.gpsimd
    batch, d_head_inner, d_head_outer, n_ctx = out_ap.shape
    d_head = d_head_outer * d_head_inner
    ncn = in_ap.shape[3]
    batch_step = in_ap.ap[1][0] // ncn
    dtype_size = mybir.dt.size(out_ap.dtype)
    dho_stride_bytes = out_ap.ap[2][0] * dtype_size
    batch_stride_bytes = out_ap.ap[0][0] * dtype_size
    if ncn & (ncn - 1) == 0:
        ncn_log2, ncn_raw = ncn.bit_length() - 1, 0
    else:
        ncn_log2, ncn_raw = 0, ncn
    inst = g.add_instruction(
        mybir.InstKVWritebackAnt(
            name=nc.get_next_instruction_name(),
            ins=[g.lower_ap(in_ap), g.lower_ap(ctx_idxs_ap)],
            outs=[*g.lower_ap_dma(out_ap.opt([0]), for_custom_bir_dma=True)],
            batch=batch,
            batch_step=batch_step,
            ncn=ncn_log2,
            ncn_raw=ncn_raw,
            d_head=d_head // 128,
            wraparound=False,
            n_ctx=n_ctx,
            gen_mode=1,
            dho_stride_bytes=dho_stride_bytes,
            batch_stride_bytes=batch_stride_bytes,
            queue_num=queue_num,
        )
    )
    return g._track_prepare_only(inst, queue_num)


def _build_program():
    nc = bacc.Bacc(
        "TRN2",
        target_bir_lowering=False,
        debug=False,
        num_devices=N_CORES,
    )
    o_d = nc.dram_tensor("o", [NIMG, H, W], F32, kind="ExternalInput").ap()
    d_d = nc.dram_tensor("d", [NIMG, H, W], F32, kind="ExternalInput").ap()
    # packed uint8 masks: cols 0:48 row mask [(img, c, j)], 48:96 col mask
    msk_d = nc.dram_tensor("msk", [P, MW], U8, kind="ExternalInput").ap()
    acc_d = nc.dram_tensor("acc", [1, P, 1, NCOLS], F32, kind="ExternalOutput").ap()

    # DRAM views: image rows split as y = c*128 + p  ->  [img, p, c, x]
    o_r = o_d.rearrange("n (c p) x -> n p c x", p=P)
    d_r = d_d.rearrange("n (c p) x -> n p c x", p=P)

    with tile.TileContext(nc) as tc, ExitStack() as ctx:
        io_pool = ctx.enter_context(tc.tile_pool(name="io", bufs=2))
        o3_pool = ctx.enter_context(tc.tile_pool(name="o3", bufs=1))
        d3_pool = ctx.enter_context(tc.tile_pool(name="d3", bufs=1))
        mask_pool = ctx.enter_context(tc.tile_pool(name="mask", bufs=1))
        work_pool = ctx.enter_context(tc.tile_pool(name="work", bufs=2))
        psum_pool = ctx.enter_context(tc.tile_pool(name="psum", bufs=2, space="PSUM"))
        acc_pool = ctx.enter_context(tc.tile_pool(name="acc", bufs=1))

        acc = acc_pool.tile([P, NCOLS], F32)
        nc.vector.memset(acc[:], 0.0)
        ones_t = acc_pool.tile([P, 1], F32)
        nc.vector.memset(ones_t[:], 1.0)
        ctxi = acc_pool.tile([P, 1], mybir.dt.int32)
        nc.gpsimd.memset(ctxi[:], 0)
        msk_u8 = mask_pool.tile([P, MW], U8)
        msk_f = mask_pool.tile([P, MW], F32)

        def rm(img, cy):
            c0 = img * NCH * NB + cy * NB
            return msk_f[:, c0 : c0 + NB]

        def cm(img):
            c0 = NIMG * NCH * NB + img * NCH * NB
            return msk_f[:, c0 : c0 + NCH * NB]

        def box_work(img, chunk):
            """chunk(cy, xs) -> [128, |xs|] AP of o rows cy*128+p, cols xs."""
            ps = psum_pool.tile([P, NCH * NB], F32, tag="ps")
            for cx in range(NCH):
                for cy in range(NCH):
                    nc.tensor.matmul(
                        ps[:, cx * NB : (cx + 1) * NB],
                        lhsT=chunk(cy, slice(cx * P, (cx + 1) * P)),
                        rhs=rm(img, cy),
                        start=(cy == 0),
                        stop=(cy == NCH - 1),
                    )
            masked_t = work_pool.tile([P, NCH * NB], F32, tag="masked")
            nc.vector.tensor_tensor(
                out=masked_t[:], in0=ps[:], in1=cm(img), op=mybir.AluOpType.mult
            )
            ps2 = psum_pool.tile([1, NCH * NB], F32, tag="ps2")
            nc.tensor.matmul(
                ps2[:], lhsT=ones_t[:], rhs=masked_t[:], start=True, stop=True
            )
            col0 = 2 * NDIFF + img * NCH * NB
            nc.vector.tensor_copy(acc[0:1, col0 : col0 + NCH * NB], ps2[:])

        def diff_work(o_ap, d_ap, col, sq_eng, tag, bufs=None):
            """stt diff (DVE) + square, each accumulating into acc columns."""
            shp = list(o_ap.shape)
            diff_t = work_pool.tile(shp, F32, tag="diff" + tag, bufs=bufs,
                                    name="diff" + tag)
            nc.vector.scalar_tensor_tensor(
                out=diff_t[:],
                in0=o_ap,
                scalar=0.0,
                in1=d_ap,
                op0=mybir.AluOpType.bypass,
                op1=mybir.AluOpType.subtract,
                accum_out=acc[:, col : col + 1],
            )
            sq_t = work_pool.tile(shp, F32, tag="sq" + tag, bufs=bufs,
                                  name="sq" + tag)
            scol = NDIFF + col
            if sq_eng == "dve":
                nc.vector.scalar_tensor_tensor(
                    out=sq_t[:],
                    in0=diff_t[:],
                    scalar=0.0,
                    in1=diff_t[:],
                    op0=mybir.AluOpType.bypass,
                    op1=mybir.AluOpType.mult,
                    accum_out=acc[:, scol : scol + 1],
                )
            else:
                nc.scalar.activation(
                    sq_t[:],
                    diff_t[:],
                    mybir.ActivationFunctionType.Square,
                    accum_out=acc[:, scol : scol + 1],
                )

        # ---- images 0..2: half-image o/d pairs ----
        HC = NCH // 2
        for img in range(NIMG - 1):
            halves = []
            for h in range(2):
                o_t = io_pool.tile([P, HC, W], F32, tag=f"o{h}", name=f"o{h}")
                nc.sync.dma_start(o_t[:], o_r[img, :, h * HC : (h + 1) * HC])
                if img == 0 and h == 0:
                    # small mask DMA tucked behind the first transfer
                    nc.sync.dma_start(msk_u8[:], msk_d)
                d_t = io_pool.tile([P, HC, W], F32, tag=f"d{h}", name=f"d{h}")
                nc.sync.dma_start(d_t[:], d_r[img, :, h * HC : (h + 1) * HC])
                if img == 0 and h == 0:
                    nc.vector.tensor_copy(msk_f[:], msk_u8[:])
                diff_work(o_t[:], d_t[:], 2 * img + h, "act", tag="h")
                halves.append(o_t)
            box_work(img, lambda cy, xs: halves[cy // HC][:, cy % HC, xs])

        # ---- image 3: o quarters (low-work zone), then tapered d pieces ----
        img = NIMG - 1
        o3 = []
        for c in range(NCH):
            t = o3_pool.tile([P, 1, W], F32, tag=f"o3c{c}", name=f"o3c{c}")
            nc.sync.dma_start(t[:], o_r[img, :, c : c + 1])
            o3.append(t)
        box_work(img, lambda cy, xs: o3[cy][:, 0, xs])

        for i, (c, x0, x1, sq_eng) in enumerate(D3_PIECES):
            d_t = d3_pool.tile([P, x1 - x0], F32, tag=f"d3p{i}", name=f"d3p{i}")
            nc.sync.dma_start(d_t[:], d_r[img, :, c, x0:x1])
            diff_work(
                o3[c][:, 0, x0:x1],
                d_t[:],
                2 * (NIMG - 1) + i,
                sq_eng,
                tag=f"p{i}",
                bufs=1,
            )

        # ---- output: SWDGE writeback prepared now, fired by the trigger ----
        _emit_wb_prep(
            nc,
            acc_d,
            acc[:].rearrange("p (a b c) -> p a b c", a=1, b=1),
            ctxi[:],
        )
        nc.gpsimd.trigger_dma(count=None)

    nc.compile()
    return nc


def _get_program():
    global _PROG
    if _PROG is None:
        _PROG = _build_program()
    return _PROG


def _prep_inputs(output, density_map, bboxes):
    o = np.ascontiguousarray(np.asarray(output, dtype=np.float32).reshape(B, H, W))
    dm = np.ascontiguousarray(
        np.asarray(density_map, dtype=np.float32).reshape(B, H, W)
    )
    bb = np.clip(np.asarray(bboxes).astype(np.int64), 0, W).astype(np.int32)
    x1, y1, x2, y2 = bb[..., 0], bb[..., 1], bb[..., 2], bb[..., 3]
    x2 = np.maximum(x2, x1)
    y2 = np.maximum(y2, y1)

    ar = np.arange(H, dtype=np.int32)
    # rm[b, y, j] = 1 if y1 <= y < y2, packed as [b, y%128, (y//128, j)]
    rmk = (
        (ar[None, :, None] >= y1[:, None, :]) & (ar[None, :, None] < y2[:, None, :])
    ).astype(np.uint8)
    rmk = rmk.reshape(B, NCH, P, NB).transpose(0, 2, 1, 3)  # [B, P, NCH, NB]
    # cm[b, j, x] = 1 if x1 <= x < x2, packed as [b, x%128, (x//128, j)]
    cmk = (
        (ar[None, None, :] >= x1[:, :, None]) & (ar[None, None, :] < x2[:, :, None])
    ).astype(np.uint8)
    cmk = cmk.reshape(B, NB, NCH, P).transpose(0, 3, 2, 1)  # [B, P, NCH, NB]
    # per-core mask tensor [P, 96]: [(img, c, j)] row-mask | col-mask blocks
    rmk = rmk.reshape(N_CORES, NIMG, P, NCH * NB).transpose(0, 2, 1, 3)
    cmk = cmk.reshape(N_CORES, NIMG, P, NCH * NB).transpose(0, 2, 1, 3)
    msk = np.ascontiguousarray(
        np.concatenate(
            [rmk.reshape(N_CORES, P, -1), cmk.reshape(N_CORES, P, -1)], axis=2
        )
    )  # [N_CORES, P, 96] uint8
    return o, dm, msk


def kernel(output, density_map, bboxes, num_objects):
    o, dm, msk = _prep_inputs(output, density_map, bboxes)

    nc = _get_program()
    in_maps = [
        {
            "o": o[i * NIMG : (i + 1) * NIMG],
            "d": dm[i * NIMG : (i + 1) * NIMG],
            "msk": msk[i],
        }
        for i in range(N_CORES)
    ]
    res = run_bass_kernel_spmd(nc, in_maps, core_ids=list(range(N_CORES)))

    per_img_d = []
    sq_total = 0.0
    box_sums = []
    for r in res.results:
        a = r["acc"].reshape(P, NCOLS)
        dcols = a[:, :NDIFF].sum(axis=0, dtype=np.float64)
        per_img = [dcols[2 * i] + dcols[2 * i + 1] for i in range(NIMG - 1)]
        per_img.append(dcols[2 * (NIMG - 1) :].sum())
        per_img_d.extend(per_img)
        sq_total += a[:, NDIFF : 2 * NDIFF].sum(dtype=np.float64)
        box_sums.append(
            a[0, 2 * NDIFF :]
            .reshape(NIMG, NCH, NB)
            .sum(axis=1, dtype=np.float64)
            .reshape(-1)
        )
    per_img_d = np.array(per_img_d)
    box_sums = np.concatenate(box_sums)

    dmap_loss = sq_total / float(num_objects)
    count_loss = float(np.mean(per_img_d**2))
    min_count = float(np.maximum(0.0, 1.0 - box_sums).sum())
    return np.array([dmap_loss, count_loss, min_count], dtype=np.float32)
